# revision 17
# baseline (speedup 1.0000x reference)
"""Trainium2 Bass kernel for nn_CorrelationFilter (SiamFC-style correlation).

Math (per batch pair b):
    out[b, oi, oj] = sum_{di<6, dj<6, c<256} x[b, oi+di, oj+dj, c] * z[b, di, dj, c]
                     + sum_{c<256} bias[0, oi, oj, b*256 + c]
with x: [B,22,22,256], z: [B,6,6,256], bias: [1,17,17,B*256], out: [B,17,17,1].

Strategy: pure data parallelism over batch across 8 NeuronCores (16 batches
per core), no cross-core communication. Host does sharding + layout prep
(transpose to channel-major, cast to bf16, and pre-reduction of the bias
over its C axis -- the bias enters the output only through sum_c); all
correlation arithmetic runs on device.

Device scheme (per core, b = 0..15):
  stage 1  -- per batch, 2 accumulating matmuls (one per 128-channel chunk):
              lhsT = zT[ch][128, 36], rhs = xT[ch][128, 484]
              -> PSUM E_b[g, p] = sum_c z[b, di, dj, c] * x[b, p2d, c],
              g = 6*di + dj, p = 22-wide flattened search position.
              xT streams through a 2-deep chunk pool: the DMA engines serve
              outstanding descriptors round-robin, so back-pressure (chunk k
              waits on chunk k-2's consumers) is what makes early batches'
              data arrive early.
  evac     -- E_b -> SBUF bf16, two batches per tile side by side in the
              free dim (e[36, 2*484]; engine writes must start at 32-aligned
              partition bases, so stacking pairs by partitions is illegal),
              ScalarE for even b / VectorE for odd b.
  bounce   -- ONE DMA per pair SBUF -> DRAM: batch B lands at element offset
              17556 = 6 * di-stride, which makes (batch, di) collapse into a
              single affine level; then ONE 3-dim gather per pair reads
              t2[u = b3*36 + g, o] = E_b[g, o + sh(g)] (sh = 22*di + dj,
              expressible only in a DRAM-side access pattern) for 72 rows.
  fold     -- one matmul per 2-batch group with a constant block-ones
              lhsT [72, 16]: fps[b, o] += sum_g t2[b3*36+g, o]; the
              36-plane shifted fold runs on the TensorE, accumulated
              across groups in a single PSUM bank.
  final    -- VectorE adds the host-reduced bias and writes [16, 17, 17].

kernel(**inputs) takes FULL unsharded inputs, returns the full output.
"""

import os
import numpy as np
import ml_dtypes

import concourse.bass as bass
import concourse.mybir as mybir
from concourse import bacc
from concourse.tile import TileContext

B, H, W, C = 128, 22, 22, 256
HZ, WZ = 6, 6
HO, WO = 17, 17
OO = HO * WO               # 289 dense output positions
NCORES = 8
BPC = B // NCORES          # 16 batches per core
P = H * W                  # 484 flattened search positions
O22 = (HO - 1) * W + WO    # 369: output span in 22-wide layout
G = HZ * WZ                # 36 correlation planes per batch
NGRP = BPC // 2            # 8 fold groups of 2 batches (= evac pairs)
DIS = HZ * P + W           # 2926: di stride in the scr gather
BOFF = HZ * DIS            # 17556: batch B's scr offset (collapses (b3, di))

FOLD_DELAY = int(os.environ.get("KERNEL_FOLD_DELAY", "3"))

_BF16 = mybir.dt.bfloat16
_F32 = mybir.dt.float32


def _grp_batches(s):
    return (2 * s, 2 * s + 2)


def build_module():
    nc = bacc.Bacc()
    xt_d = nc.dram_tensor("xt", [2, 128, BPC, P], _BF16, kind="ExternalInput")
    zt_d = nc.dram_tensor("zt", [128, 2, BPC, G], _BF16, kind="ExternalInput")
    bq_d = nc.dram_tensor("bq", [BPC, OO], _F32, kind="ExternalInput")
    fc_d = nc.dram_tensor("fc", [2 * G, NGRP, BPC], _BF16, kind="ExternalInput")
    out_d = nc.dram_tensor("out", [BPC, HO, WO], _F32, kind="ExternalOutput")

    with TileContext(nc) as tc:
        with (
            tc.tile_pool(name="const", bufs=1) as cpool,
            tc.tile_pool(name="xtp", bufs=2) as xpool,
            tc.tile_pool(name="evac", bufs=4) as epool,
            tc.tile_pool(name="t2p", bufs=3) as tpool,
            tc.tile_pool(name="work", bufs=1) as work,
            tc.tile_pool(name="qps", bufs=7, space="PSUM") as qpool,
            tc.tile_pool(name="fps", bufs=1, space="PSUM") as fpool,
            tc.tile_pool(name="dram", bufs=1, space="DRAM") as dpool,
        ):
            # fold stationary: FC[u, s, b] = 1 iff batch b is row-block u//36
            # of fold group s (host-built: engine memsets need 32-aligned
            # partition bases, which the 36-row blocks don't have)
            FC = cpool.tile([2 * G, NGRP, BPC], _BF16, name="fc")
            nc.scalar.dma_start(out=FC[:], in_=fc_d[:])

            zt_t = cpool.tile([128, 2, BPC, G], _BF16, name="ztt")
            # zt first on the sync queue: it gates the first matmul
            nc.sync.dma_start(out=zt_t[:], in_=zt_d[:])
            bq_t = cpool.tile([BPC, OO], _F32, name="bqt")

            # DRAM bounce scratch per pair: batch A planes at 0, batch B at
            # BOFF (flat element offsets; B straddles row boundaries so the
            # gather's (b3, di) levels share one stride)
            scrs = [
                dpool.tile([BOFF + G * P], _BF16, name=f"scr{s}")
                for s in range(NGRP)
            ]

            # fold accumulator rows 0:16; 374 cols so the final (i, j) view
            # can use 22-wide row strides
            fps = fpool.tile([32, HO * W], _F32, name="fps")

            def emit_fold(s):
                t2 = tpool.tile([2 * G, O22], _BF16, name="t2", tag="t2")
                sap = scrs[s][:]
                nc.gpsimd.dma_start(
                    out=t2[:],
                    in_=bass.AP(
                        sap.tensor, sap.offset,
                        [[DIS, 2 * HZ], [P + 1, WZ], [1, O22]],
                    ),
                )
                nc.tensor.matmul(
                    fps[0:BPC, 0:O22],
                    FC[:, s, :],
                    t2[:],
                    start=(s == 0),
                    stop=(s == NGRP - 1),
                )

            emitted = 0
            xtile = [None, None]
            for b in range(BPC):
                if b % 4 == 0:
                    for ch in range(2):
                        xtile[ch] = xpool.tile(
                            [128, 4, P], _BF16, name=f"xt{ch}_{b // 4}",
                            tag=f"xt{ch}",
                        )
                        nc.sync.dma_start(
                            out=xtile[ch][:],
                            in_=xt_d[ch, :, b : b + 4, :],
                        )
                if b == 1:
                    # bias only needed at the very end; dispatch late so it
                    # never competes with zt/xt head traffic
                    nc.scalar.dma_start(out=bq_t[:], in_=bq_d[:])
                q = qpool.tile([64, P], _F32, name="q", tag="q")
                nc.tensor.matmul(
                    q[0:G, :], zt_t[:, 0, b, :], xtile[0][:, b % 4, :],
                    start=True, stop=False,
                )
                nc.tensor.matmul(
                    q[0:G, :], zt_t[:, 1, b, :], xtile[1][:, b % 4, :],
                    start=False, stop=True,
                )
                if b % 2 == 0:
                    e = epool.tile([G, 2, P], _BF16, name="e", tag="e")
                    nc.scalar.copy(out=e[:, 0, :], in_=q[0:G, :])
                else:
                    nc.vector.tensor_copy(out=e[:, 1, :], in_=q[0:G, :])
                    sap = scrs[b // 2][:]
                    nc.gpsimd.dma_start(
                        out=bass.AP(
                            sap.tensor, sap.offset,
                            [[P, G], [BOFF, 2], [1, P]],
                        ),
                        in_=e[:],
                    )
                # emit fold groups a few batches after their data is bounced
                # so no engine queue stalls long on the bounce chain
                while emitted < NGRP and 2 * emitted + 1 + FOLD_DELAY <= b:
                    emit_fold(emitted)
                    emitted += 1
            while emitted < NGRP:
                emit_fold(emitted)
                emitted += 1

            outb = work.tile([BPC, HO, WO], _F32, name="outb")
            acc_v = fps[0:BPC, :].rearrange("b (i j) -> b i j", j=W)[:, :, 0:WO]
            bias_v = bq_t[:].rearrange("b (i j) -> b i j", j=WO)
            nc.vector.tensor_add(out=outb[:], in0=acc_v, in1=bias_v)
            nc.sync.dma_start(out=out_d[:], in_=outb[:])

    nc.compile()
    return nc


def prep_inputs(x, z, b):
    """Host-side shard + layout prep. Returns per-core in_maps."""
    xb = np.asarray(x).astype(ml_dtypes.bfloat16)
    zb = np.asarray(z).astype(ml_dtypes.bfloat16)
    # bias enters the output only via sum over its C axis; reduce on host
    bred = np.asarray(b, dtype=np.float32).reshape(OO, B, C).sum(axis=2)  # [289, B]
    fc = np.zeros((2 * G, NGRP, BPC), dtype=ml_dtypes.bfloat16)
    for s in range(NGRP):
        b0g, b1g = _grp_batches(s)
        for u3 in range(b1g - b0g):
            fc[G * u3 : G * u3 + G, s, b0g + u3] = 1.0
    in_maps = []
    for core in range(NCORES):
        b0 = core * BPC
        xs = xb[b0 : b0 + BPC].reshape(BPC, P, C)
        xT = np.ascontiguousarray(xs.transpose(2, 0, 1)).reshape(2, 128, BPC, P)
        zs = zb[b0 : b0 + BPC].reshape(BPC, G, 2, 128)
        zT = np.ascontiguousarray(zs.transpose(3, 2, 0, 1))  # [128, 2, BPC, G]
        bq = np.ascontiguousarray(bred[:, b0 : b0 + BPC].T)  # [BPC, 289]
        in_maps.append({"xt": xT, "zt": zT, "bq": bq, "fc": fc})
    return in_maps


_cache = {}


def _ensure_ntff_hook():
    """The axon NTFF profile hook normally lives in antenv.axon_hooks, which
    this image lacks; synthesize it from the boot shim's ctypes wrapper."""
    try:
        from antenv.axon_hooks import get_axon_ntff_profile_hook  # noqa: F401
        return True
    except ImportError:
        pass
    try:
        import sys, types
        from trn_agent_boot.trn_boot import _ntff_profile_via_ctypes

        so = os.environ.get("AXON_PJRT_SO", "/opt/axon/libaxon_pjrt.so")
        hook = _ntff_profile_via_ctypes(so)
        mod = types.ModuleType("antenv.axon_hooks")
        mod.get_axon_ntff_profile_hook = lambda: hook
        mod.set_axon_ntff_profile_hook = lambda h: None
        sys.modules["antenv.axon_hooks"] = mod
        import antenv

        antenv.axon_hooks = mod
        return True
    except Exception:
        return False


def kernel(x, z, b):
    from concourse.bass_utils import run_bass_kernel_spmd

    if "nc" not in _cache:
        _cache["nc"] = build_module()
    nc = _cache["nc"]
    in_maps = prep_inputs(x, z, b)
    trace = bool(int(os.environ.get("KERNEL_TRACE", "0") or 0))
    if trace:
        trace = _ensure_ntff_hook()
    res = run_bass_kernel_spmd(
        nc,
        in_maps,
        core_ids=list(range(NCORES)),
        trace=trace,
    )
    _cache["last_result"] = res
    out = np.concatenate([r["out"].reshape(BPC, HO, WO) for r in res.results], axis=0)
    return out[..., None].astype(np.float32)


# revision 25
# speedup vs baseline: 1.0702x; 1.0702x over previous
"""Trainium2 Bass kernel for nn_CorrelationFilter (SiamFC-style correlation).

Math (per batch pair b):
    out[b, oi, oj] = sum_{di<6, dj<6, c<256} x[b, oi+di, oj+dj, c] * z[b, di, dj, c]
                     + sum_{c<256} bias[0, oi, oj, b*256 + c]
with x: [B,22,22,256], z: [B,6,6,256], bias: [1,17,17,B*256], out: [B,17,17,1].

Strategy: pure data parallelism over batch across 8 NeuronCores (16 batches
per core), no cross-core communication. Host does sharding + layout prep
(transpose to channel-major, cast to bf16, and pre-reduction of the bias
over its C axis -- the bias enters the output only through sum_c); all
correlation arithmetic runs on device.

Device scheme (per core, b = 0..15):
  stage 1  -- per batch, 2 accumulating matmuls (one per 128-channel chunk):
              lhsT = zT[ch][128, 36], rhs = xT[ch][128, 484]
              -> PSUM E_b[g, p] = sum_c z[b, di, dj, c] * x[b, p2d, c],
              g = 6*di + dj, p = 22-wide flattened search position.
              Adjacent batches run on disjoint PE column groups
              (tile_position (0,0) / (0,64), PSUM rows [0:36] / [64:100]),
              so their matmul streams overlap in the array -> ~2x stage-1.
              xT streams through a 3-deep chunk pool: the DMA engines serve
              outstanding descriptors round-robin, so back-pressure (chunk k
              waits on chunk k-3's consumers) bounds how much later traffic
              can starve the next-needed chunk.
  evac     -- E_b -> SBUF bf16 at the same partition base as its PSUM rows
              (engine ops cannot cross partition bases), ScalarE for even b
              / VectorE for odd b.
  bounce   -- one DMA per batch SBUF -> DRAM: batch B of a pair lands at
              element offset 17556 = 6 * di-stride, which makes (batch, di)
              collapse into a single affine level; then ONE 3-dim gather per
              pair reads t2[u = b3*36 + g, o] = E_b[g, o + sh(g)]
              (sh = 22*di + dj, expressible only in a DRAM-side access
              pattern) for 72 rows.
  fold     -- one matmul per 2-batch group with a constant block-ones
              lhsT [72, 16]: fps[b, o] += sum_g t2[b3*36+g, o]; the
              36-plane shifted fold runs on the TensorE, accumulated
              across groups in a single PSUM bank.
  final    -- VectorE adds the host-reduced bias and writes [16, 17, 17].

kernel(**inputs) takes FULL unsharded inputs, returns the full output.
"""

import os
import numpy as np
import ml_dtypes

import concourse.bass as bass
import concourse.mybir as mybir
from concourse import bacc
from concourse.tile import TileContext

B, H, W, C = 128, 22, 22, 256
HZ, WZ = 6, 6
HO, WO = 17, 17
OO = HO * WO               # 289 dense output positions
NCORES = 8
BPC = B // NCORES          # 16 batches per core
P = H * W                  # 484 flattened search positions
O22 = (HO - 1) * W + WO    # 369: output span in 22-wide layout
G = HZ * WZ                # 36 correlation planes per batch
NGRP = BPC // 2            # 8 fold groups of 2 batches (= evac pairs)
DIS = HZ * P + W           # 2926: di stride in the scr gather
BOFF = HZ * DIS            # 17556: batch B's scr offset (collapses (b3, di))

FOLD_DELAY = int(os.environ.get("KERNEL_FOLD_DELAY", "3"))

_BF16 = mybir.dt.bfloat16
_F32 = mybir.dt.float32


def _grp_batches(s):
    return (2 * s, 2 * s + 2)


def build_module():
    nc = bacc.Bacc()
    xt_d = nc.dram_tensor("xt", [2, 128, BPC, P], _BF16, kind="ExternalInput")
    zt_d = nc.dram_tensor("zt", [128, 2, BPC, G], _BF16, kind="ExternalInput")
    bq_d = nc.dram_tensor("bq", [BPC, OO], _F32, kind="ExternalInput")
    fc_d = nc.dram_tensor("fc", [2 * G, NGRP, BPC], _BF16, kind="ExternalInput")
    out_d = nc.dram_tensor("out", [BPC, HO, WO], _F32, kind="ExternalOutput")

    with TileContext(nc) as tc:
        with (
            tc.tile_pool(name="const", bufs=1) as cpool,
            tc.tile_pool(name="xtp", bufs=3) as xpool,
            tc.tile_pool(name="evac", bufs=3) as epool,
            tc.tile_pool(name="t2p", bufs=3) as tpool,
            tc.tile_pool(name="work", bufs=1) as work,
            tc.tile_pool(name="qps", bufs=7, space="PSUM") as qpool,
            tc.tile_pool(name="fps", bufs=1, space="PSUM") as fpool,
            tc.tile_pool(name="dram", bufs=1, space="DRAM") as dpool,
        ):
            # fold stationary: FC[u, s, b] = 1 iff batch b is row-block u//36
            # of fold group s (host-built: engine memsets need 32-aligned
            # partition bases, which the 36-row blocks don't have)
            FC = cpool.tile([2 * G, NGRP, BPC], _BF16, name="fc")
            nc.scalar.dma_start(out=FC[:], in_=fc_d[:])

            zt_t = cpool.tile([128, 2, BPC, G], _BF16, name="ztt")
            # zt first on the sync queue: it gates the first matmul
            nc.sync.dma_start(out=zt_t[:], in_=zt_d[:])
            bq_t = cpool.tile([BPC, OO], _F32, name="bqt")

            # DRAM bounce scratch per pair: batch A planes at 0, batch B at
            # BOFF (flat element offsets; B straddles row boundaries so the
            # gather's (b3, di) levels share one stride)
            scrs = [
                dpool.tile([BOFF + G * P], _BF16, name=f"scr{s}")
                for s in range(NGRP)
            ]

            # fold accumulator rows 0:16; 374 cols so the final (i, j) view
            # can use 22-wide row strides
            fps = fpool.tile([128, HO * W], _F32, name="fps")

            def emit_fold(s):
                t2 = tpool.tile([2 * G, O22], _BF16, name="t2", tag="t2")
                sap = scrs[s][:]
                # reads go on the Sync queue (idle once xt is dispatched);
                # writes live on GpSimd, keeping each queue short
                nc.sync.dma_start(
                    out=t2[:],
                    in_=bass.AP(
                        sap.tensor, sap.offset,
                        [[DIS, 2 * HZ], [P + 1, WZ], [1, O22]],
                    ),
                )
                # column group 96 keeps folds off stage-1's groups (0 / 64)
                nc.tensor.matmul(
                    fps[96 : 96 + BPC, 0:O22],
                    FC[:, s, :],
                    t2[:],
                    start=(s == 0),
                    stop=(s == NGRP - 1),
                    tile_position=(0, 96),
                )

            emitted = 0
            xtile = [None, None]
            for b in range(BPC):
                if b % 4 == 0:
                    for ch in range(2):
                        xtile[ch] = xpool.tile(
                            [128, 4, P], _BF16, name=f"xt{ch}_{b // 4}",
                            tag=f"xt{ch}",
                        )
                        nc.sync.dma_start(
                            out=xtile[ch][:],
                            in_=xt_d[ch, :, b : b + 4, :],
                        )
                if b == 1:
                    # bias only needed at the very end; dispatch late so it
                    # never competes with zt/xt head traffic
                    nc.scalar.dma_start(out=bq_t[:], in_=bq_d[:])
                # adjacent batches on disjoint PE column groups so their
                # streams overlap; PSUM rows track the column group
                pbase = 0 if b % 2 == 0 else 64
                q = qpool.tile([128, P], _F32, name="q", tag="q")
                nc.tensor.matmul(
                    q[pbase : pbase + G, :], zt_t[:, 0, b, :],
                    xtile[0][:, b % 4, :],
                    start=True, stop=False, tile_position=(0, pbase),
                )
                nc.tensor.matmul(
                    q[pbase : pbase + G, :], zt_t[:, 1, b, :],
                    xtile[1][:, b % 4, :],
                    start=False, stop=True, tile_position=(0, pbase),
                )
                sap = scrs[b // 2][:]
                if b % 2 == 0:
                    e = epool.tile([64, P], _BF16, name="ea", tag="ea")
                    nc.scalar.copy(out=e[0:G, :], in_=q[0:G, :])
                    nc.gpsimd.dma_start(
                        out=bass.AP(sap.tensor, sap.offset, [[P, G], [1, P]]),
                        in_=e[0:G, :],
                    )
                else:
                    e = epool.tile([128, P], _BF16, name="eb", tag="eb")
                    nc.vector.tensor_copy(
                        out=e[64 : 64 + G, :], in_=q[64 : 64 + G, :]
                    )
                    nc.gpsimd.dma_start(
                        out=bass.AP(
                            sap.tensor, sap.offset + BOFF, [[P, G], [1, P]]
                        ),
                        in_=e[64 : 64 + G, :],
                    )
                # emit fold groups a few batches after their data is bounced
                # so no engine queue stalls long on the bounce chain
                while emitted < NGRP and 2 * emitted + 1 + FOLD_DELAY <= b:
                    emit_fold(emitted)
                    emitted += 1
            while emitted < NGRP:
                emit_fold(emitted)
                emitted += 1

            outb = work.tile([BPC, HO, WO], _F32, name="outb")
            acc_v = fps[96 : 96 + BPC, :].rearrange(
                "b (i j) -> b i j", j=W
            )[:, :, 0:WO]
            bias_v = bq_t[:].rearrange("b (i j) -> b i j", j=WO)
            nc.vector.tensor_add(out=outb[:], in0=acc_v, in1=bias_v)
            nc.sync.dma_start(out=out_d[:], in_=outb[:])

    nc.compile()
    return nc


def prep_inputs(x, z, b):
    """Host-side shard + layout prep. Returns per-core in_maps."""
    xb = np.asarray(x).astype(ml_dtypes.bfloat16)
    zb = np.asarray(z).astype(ml_dtypes.bfloat16)
    # bias enters the output only via sum over its C axis; reduce on host
    bred = np.asarray(b, dtype=np.float32).reshape(OO, B, C).sum(axis=2)  # [289, B]
    fc = np.zeros((2 * G, NGRP, BPC), dtype=ml_dtypes.bfloat16)
    for s in range(NGRP):
        b0g, b1g = _grp_batches(s)
        for u3 in range(b1g - b0g):
            fc[G * u3 : G * u3 + G, s, b0g + u3] = 1.0
    in_maps = []
    for core in range(NCORES):
        b0 = core * BPC
        xs = xb[b0 : b0 + BPC].reshape(BPC, P, C)
        xT = np.ascontiguousarray(xs.transpose(2, 0, 1)).reshape(2, 128, BPC, P)
        zs = zb[b0 : b0 + BPC].reshape(BPC, G, 2, 128)
        zT = np.ascontiguousarray(zs.transpose(3, 2, 0, 1))  # [128, 2, BPC, G]
        bq = np.ascontiguousarray(bred[:, b0 : b0 + BPC].T)  # [BPC, 289]
        in_maps.append({"xt": xT, "zt": zT, "bq": bq, "fc": fc})
    return in_maps


_cache = {}


def _ensure_ntff_hook():
    """The axon NTFF profile hook normally lives in antenv.axon_hooks, which
    this image lacks; synthesize it from the boot shim's ctypes wrapper."""
    try:
        from antenv.axon_hooks import get_axon_ntff_profile_hook  # noqa: F401
        return True
    except ImportError:
        pass
    try:
        import sys, types
        from trn_agent_boot.trn_boot import _ntff_profile_via_ctypes

        so = os.environ.get("AXON_PJRT_SO", "/opt/axon/libaxon_pjrt.so")
        hook = _ntff_profile_via_ctypes(so)
        mod = types.ModuleType("antenv.axon_hooks")
        mod.get_axon_ntff_profile_hook = lambda: hook
        mod.set_axon_ntff_profile_hook = lambda h: None
        sys.modules["antenv.axon_hooks"] = mod
        import antenv

        antenv.axon_hooks = mod
        return True
    except Exception:
        return False


def kernel(x, z, b):
    from concourse.bass_utils import run_bass_kernel_spmd

    if "nc" not in _cache:
        _cache["nc"] = build_module()
    nc = _cache["nc"]
    in_maps = prep_inputs(x, z, b)
    trace = bool(int(os.environ.get("KERNEL_TRACE", "0") or 0))
    if trace:
        trace = _ensure_ntff_hook()
    res = run_bass_kernel_spmd(
        nc,
        in_maps,
        core_ids=list(range(NCORES)),
        trace=trace,
    )
    _cache["last_result"] = res
    out = np.concatenate([r["out"].reshape(BPC, HO, WO) for r in res.results], axis=0)
    return out[..., None].astype(np.float32)
